# revision 33
# baseline (speedup 1.0000x reference)
"""CE top-k loss kernel for Trainium2 (raw Bass), data-parallel over batch on 8 cores.

Math: the reference scatters the global max of x into the label column, argsorts
each row ascending, drops the top-k entries, and computes
    loss = mean_b log( sum_{j in bottom M-k} exp(x[b,j] - x[b,y[b]]) + 1 ).
Because the label column is forced to the global max, the excluded top-k set is
exactly {label} U {top (k-1) non-label values}, so with
    S = sum_j exp(x_j - s_y)   (label term contributes exp(0) = 1 exactly)
    T = sum of the top (k-1) non-label exp values
    loss_row = log(S - 1 - T + 1) = log(S - T).
No sort needed. Per 128-row shard each core streams x once:
  DMA tile -> ACT exp(x - s_y) (bf16 out, fp32 row-sum accumulator)
           -> DVE top-8 of the exp tile (exp is monotone; label value == 1.0).
Tail: top-8 of per-tile candidates, match_replace one occurrence of 1.0 (the
label), re-sort, T = sum of first k-1, loss = Ln(S - T). The label logit s_y
arrives via one indirect DMA gather at host-precomputed flat offsets.

Raw Bass (not Tile): this toolchain's codegen encodes at most ONE sync wait per
instruction, and Tile attaches one wait per tracked dependency (it is not
transitively minimal), which is unencodable here. With explicit semaphores each
wait_ge is a standalone event-sem instruction, and same-engine program order +
transitive waits make every instruction carry <= 1 wait.
"""

from contextlib import ExitStack

import numpy as np

import concourse.bass as bass
import concourse.mybir as mybir
from concourse.bass_utils import run_bass_kernel_spmd

B = 1024
M = 50257
N_CORES = 8
BP = B // N_CORES  # 128 rows per core = one SBUF partition per row

TILE_W = 2048
NBUF = 8  # x-tile ring slots == number of round-robin DMA completion sems
SENTINEL = -2.0e38  # match-list filler; never present in the data
NEG_FILL = -1.0e30  # value used to knock the label out of the top-8 list

F32 = mybir.dt.float32
BF16 = mybir.dt.bfloat16
I32 = mybir.dt.int32


def build_program(bp: int, m: int, k: int, tile_w: int) -> bass.Bass:
    assert 0 <= k <= 8, "top-8 based tail handles k <= 8 only"
    assert m % tile_w == 0 or m % tile_w >= 8, "tail tile must be >= 8 wide for max8"
    n_tiles = (m + tile_w - 1) // tile_w
    nbuf = min(NBUF, n_tiles)

    nc = bass.Bass()
    x = nc.dram_tensor("x", [bp * m, 1], F32, kind="ExternalInput")
    # yoff[p] = p*m + y[p]: flat gather offsets, precomputed on host at shard time
    yoff = nc.dram_tensor("yoff", [bp, 1], I32, kind="ExternalInput")
    out = nc.dram_tensor("out", [bp, 1], F32, kind="ExternalOutput")
    x2d = x[:, :].rearrange("(p m) one -> p (m one)", p=bp)

    exp_f = mybir.ActivationFunctionType.Exp
    ln_f = mybir.ActivationFunctionType.Ln
    copy_f = mybir.ActivationFunctionType.Copy

    with ExitStack() as ctx:
        xt = ctx.enter_context(nc.sbuf_tensor([bp, nbuf * tile_w], F32))
        escr = ctx.enter_context(nc.sbuf_tensor([bp, 2 * tile_w], BF16))
        cand = ctx.enter_context(nc.sbuf_tensor([bp, 8 * n_tiles], BF16))
        sums = ctx.enter_context(nc.sbuf_tensor([bp, n_tiles], F32))
        idx = ctx.enter_context(nc.sbuf_tensor([bp, 1], I32))
        sy = ctx.enter_context(nc.sbuf_tensor([bp, 1], F32))
        neg_sy = ctx.enter_context(nc.sbuf_tensor([bp, 1], F32))
        top8 = ctx.enter_context(nc.sbuf_tensor([bp, 8], BF16))
        mlist = ctx.enter_context(nc.sbuf_tensor([bp, 8], BF16))
        top8r = ctx.enter_context(nc.sbuf_tensor([bp, 8], BF16))
        top8s = ctx.enter_context(nc.sbuf_tensor([bp, 8], BF16))
        tsum = ctx.enter_context(nc.sbuf_tensor([bp, 1], F32))
        s_all = ctx.enter_context(nc.sbuf_tensor([bp, 1], F32))
        diff = ctx.enter_context(nc.sbuf_tensor([bp, 1], F32))
        loss = ctx.enter_context(nc.sbuf_tensor([bp, 1], F32))

        dma_sems = [ctx.enter_context(nc.semaphore(f"dma{q}")) for q in range(nbuf)]
        sw_sem = ctx.enter_context(nc.semaphore("sw"))
        act_sem = ctx.enter_context(nc.semaphore("act"))
        dve_sem = ctx.enter_context(nc.semaphore("dve"))
        out_sem = ctx.enter_context(nc.semaphore("outd"))
        block = ctx.enter_context(nc.Block())

        def tw(t):
            return min(tile_w, m - t * tile_w)

        # dve_sem total: n_tiles maxes + tail ops (s_all reduce + branch ops)
        n_dve = n_tiles + 1 + (1 if k <= 1 else 7)

        @block.gpsimd
        def _(gpsimd):
            # s_y = x[p, y[p]] via one indirect gather
            gpsimd.dma_start(out=idx[:, :], in_=yoff[:, :]).then_inc(sw_sem, 16)
            gpsimd.wait_ge(sw_sem, 16)
            gpsimd.indirect_dma_start(
                out=sy[:, :],
                out_offset=None,
                in_=x[:, :],
                in_offset=bass.IndirectOffsetOnAxis(ap=idx[:, :1], axis=0),
            ).then_inc(sw_sem, 16)

        @block.sync
        def _(sync):
            for t in range(n_tiles):
                if t >= nbuf:
                    # slot reuse: exp(t-nbuf) must be done reading the slot;
                    # transitively this also covers the slot's previous DMA
                    sync.wait_ge(act_sem, t - nbuf + 2)
                s = (t % nbuf) * tile_w
                sync.dma_start(
                    out=xt[:, s : s + tw(t)],
                    in_=x2d[:, t * tile_w : t * tile_w + tw(t)],
                ).then_inc(dma_sems[t % nbuf], 16)
            # final store after Ln
            sync.wait_ge(act_sem, n_tiles + 2)
            sync.dma_start(out=out[:, :], in_=loss[:, :]).then_inc(out_sem, 16)
            sync.wait_ge(out_sem, 16)

        @block.scalar
        def _(scalar):
            scalar.wait_ge(sw_sem, 32)
            nc.scalar.activation(
                out=neg_sy[:, :], in_=sy[:, :], func=copy_f, bias=0.0, scale=-1.0
            ).then_inc(act_sem, 1)
            # ACT is deep-pipelined: drain so the exps' bias read sees neg_sy
            scalar.wait_ge(act_sem, 1)
            for t in range(n_tiles):
                scalar.wait_ge(dma_sems[t % nbuf], 16 * (t // nbuf + 1))
                if t >= 2:
                    # escr ping-pong: max(t-2) must be done with this half
                    scalar.wait_ge(dve_sem, t - 1)
                e = (t % 2) * tile_w
                nc.scalar.activation(
                    out=escr[:, e : e + tw(t)],
                    in_=xt[:, (t % nbuf) * tile_w : (t % nbuf) * tile_w + tw(t)],
                    func=exp_f,
                    bias=neg_sy[:, :1],
                    scale=1.0,
                    accum_out=sums[:, t : t + 1],
                ).then_inc(act_sem, 1)
            scalar.wait_ge(dve_sem, n_dve)
            nc.scalar.activation(out=loss[:, :], in_=diff[:, :], func=ln_f).then_inc(
                act_sem, 1
            )

        @block.vector
        def _(vector):
            for t in range(n_tiles):
                vector.wait_ge(act_sem, t + 2)
                e = (t % 2) * tile_w
                nc.vector.max(
                    out=cand[:, 8 * t : 8 * t + 8], in_=escr[:, e : e + tw(t)]
                ).then_inc(dve_sem, 1)

            # Tail: DVE is pipelined too, so serialize each dependent step with
            # a retire-wait (tiny ops; sems are the only ordering primitive).
            cnt = n_tiles

            def dve_op(emit):
                nonlocal cnt
                vector.wait_ge(dve_sem, cnt)
                cnt += 1
                emit().then_inc(dve_sem, 1)

            # all exps retired (max(n-1) waited act_sem >= n_tiles+1): sums ready
            dve_op(
                lambda: nc.vector.reduce_sum(
                    out=s_all[:, :], in_=sums[:, :], axis=mybir.AxisListType.X
                )
            )
            if k <= 1:
                # k=0: nothing excluded -> loss_row = log(S + 1)
                # k=1: only the label excluded -> log(S - 1 + 1) = log(S)
                if k == 0:
                    dve_op(
                        lambda: nc.vector.tensor_scalar_add(
                            diff[:, :], s_all[:, :], 1.0
                        )
                    )
                else:
                    dve_op(lambda: nc.vector.tensor_copy(diff[:, :], s_all[:, :]))
            else:
                dve_op(lambda: nc.vector.max(out=top8[:, :], in_=cand[:, :]))
                dve_op(lambda: nc.vector.memset(mlist[:, 0:1], 1.0))
                dve_op(lambda: nc.vector.memset(mlist[:, 1:8], SENTINEL))
                dve_op(
                    lambda: nc.vector.match_replace(
                        out=top8r[:, :],
                        in_to_replace=mlist[:, :],
                        in_values=top8[:, :],
                        imm_value=NEG_FILL,
                    )
                )
                dve_op(lambda: nc.vector.max(out=top8s[:, :], in_=top8r[:, :]))
                dve_op(
                    lambda: nc.vector.reduce_sum(
                        out=tsum[:, :],
                        in_=top8s[:, : k - 1],
                        axis=mybir.AxisListType.X,
                    )
                )
                dve_op(
                    lambda: nc.vector.tensor_sub(
                        out=diff[:, :], in0=s_all[:, :], in1=tsum[:, :]
                    )
                )
            assert cnt == n_dve, (cnt, n_dve)

    return nc


_program_cache: dict = {}


def _get_program(k: int) -> bass.Bass:
    if k not in _program_cache:
        _program_cache[k] = build_program(BP, M, k, TILE_W)
    return _program_cache[k]


def _run(x, y, k, **spmd_kwargs):
    x = np.asarray(x, dtype=np.float32)
    y = np.asarray(y)
    k = int(k)
    assert x.shape == (B, M), x.shape
    assert y.shape == (B,), y.shape

    nc = _get_program(k)
    in_maps = []
    for i in range(N_CORES):
        xs = np.ascontiguousarray(x[i * BP : (i + 1) * BP]).reshape(-1, 1)
        ys = y[i * BP : (i + 1) * BP].astype(np.int64)
        yo = (np.arange(BP, dtype=np.int64) * M + ys).astype(np.int32).reshape(BP, 1)
        in_maps.append({"x": xs, "yoff": yo})

    res = run_bass_kernel_spmd(nc, in_maps, list(range(N_CORES)), **spmd_kwargs)
    losses = np.concatenate(
        [np.asarray(r["out"], dtype=np.float32).reshape(BP) for r in res.results]
    )
    return np.asarray(losses.mean(dtype=np.float64), dtype=np.float32), res


def kernel(x, y, k) -> np.ndarray:
    out, _ = _run(x, y, k)
    return out


# revision 38
# speedup vs baseline: 6.2676x; 6.2676x over previous
"""CE top-k loss kernel for Trainium2 (raw Bass), data-parallel over batch on 8 cores.

Math: the reference scatters the global max of x into the label column, argsorts
each row ascending, drops the top-k entries, and computes
    loss = mean_b log( sum_{j in bottom M-k} exp(x[b,j] - x[b,y[b]]) + 1 ).
Because the label column is forced to the global max, the excluded top-k set is
exactly {label} U {top (k-1) non-label values}, so with
    S = sum_j exp(x_j - s_y)   (label term contributes exp(0) = 1 exactly)
    T = sum of the top (k-1) non-label exp values
    loss_row = log(S - 1 - T + 1) = log(S - T).
No sort needed. Per 128-row shard each core streams x once:
  DMA tile -> ACT exp(x - s_y) (bf16 out, fp32 row-sum accumulator)
           -> DVE top-8 of the exp tile (exp is monotone; label value == 1.0).
Tail: top-8 of per-tile candidates, match_replace one occurrence of 1.0 (the
label), re-sort, T = sum of first k-1, loss = Ln(S - T). The label logit s_y
arrives via one indirect DMA gather at host-precomputed flat offsets.

Raw Bass (not Tile): this toolchain's codegen encodes at most ONE sync wait per
instruction, and Tile attaches one wait per tracked dependency (it is not
transitively minimal), which is unencodable here. With explicit semaphores each
wait_ge is a standalone event-sem instruction, and same-engine program order +
transitive waits make every instruction carry <= 1 wait.
"""

from contextlib import ExitStack

import numpy as np

import concourse.bass as bass
import concourse.mybir as mybir
from concourse.bass_utils import run_bass_kernel_spmd

B = 1024
M = 50257
N_CORES = 8
BP = B // N_CORES  # 128 rows per core = one SBUF partition per row

TILE_W = 2048
NBUF = 8  # x-tile ring slots == number of round-robin DMA completion sems
SENTINEL = -2.0e38  # match-list filler; never present in the data
NEG_FILL = -1.0e30  # value used to knock the label out of the top-8 list

F32 = mybir.dt.float32
BF16 = mybir.dt.bfloat16
I32 = mybir.dt.int32


def build_program(bp: int, m: int, k: int, tile_w: int, repeat: int = 1) -> bass.Bass:
    """repeat > 1 re-streams the same data that many times (timing builds only:
    steady-state loop time = (T(R2) - T(R1)) / (R2 - R1), dispatch cancels)."""
    assert 0 <= k <= 8, "top-8 based tail handles k <= 8 only"
    assert m % tile_w == 0 or m % tile_w >= 8, "tail tile must be >= 8 wide for max8"
    n_tiles = (m + tile_w - 1) // tile_w
    nbuf = min(NBUF, n_tiles)
    n_stream = repeat * n_tiles

    nc = bass.Bass()
    x = nc.dram_tensor("x", [bp * m, 1], F32, kind="ExternalInput")
    # yoff[p] = p*m + y[p]: flat gather offsets, precomputed on host at shard time
    yoff = nc.dram_tensor("yoff", [bp, 1], I32, kind="ExternalInput")
    out = nc.dram_tensor("out", [bp, 1], F32, kind="ExternalOutput")
    x2d = x[:, :].rearrange("(p m) one -> p (m one)", p=bp)

    exp_f = mybir.ActivationFunctionType.Exp
    ln_f = mybir.ActivationFunctionType.Ln
    copy_f = mybir.ActivationFunctionType.Copy

    with ExitStack() as ctx:
        xt = ctx.enter_context(nc.sbuf_tensor([bp, nbuf * tile_w], F32))
        escr = ctx.enter_context(nc.sbuf_tensor([bp, 2 * tile_w], BF16))
        cand = ctx.enter_context(nc.sbuf_tensor([bp, 8 * n_tiles], BF16))
        sums = ctx.enter_context(nc.sbuf_tensor([bp, n_tiles], F32))
        idx = ctx.enter_context(nc.sbuf_tensor([bp, 1], I32))
        sy = ctx.enter_context(nc.sbuf_tensor([bp, 1], F32))
        neg_sy = ctx.enter_context(nc.sbuf_tensor([bp, 1], F32))
        top8 = ctx.enter_context(nc.sbuf_tensor([bp, 8], BF16))
        mlist = ctx.enter_context(nc.sbuf_tensor([bp, 8], BF16))
        top8r = ctx.enter_context(nc.sbuf_tensor([bp, 8], BF16))
        top8s = ctx.enter_context(nc.sbuf_tensor([bp, 8], BF16))
        tsum = ctx.enter_context(nc.sbuf_tensor([bp, 1], F32))
        s_all = ctx.enter_context(nc.sbuf_tensor([bp, 1], F32))
        diff = ctx.enter_context(nc.sbuf_tensor([bp, 1], F32))
        loss = ctx.enter_context(nc.sbuf_tensor([bp, 1], F32))

        dma_sems = [ctx.enter_context(nc.semaphore(f"dma{q}")) for q in range(nbuf)]
        sw_sem = ctx.enter_context(nc.semaphore("sw"))
        act_sem = ctx.enter_context(nc.semaphore("act"))
        dve_sem = ctx.enter_context(nc.semaphore("dve"))
        out_sem = ctx.enter_context(nc.semaphore("outd"))
        block = ctx.enter_context(nc.Block())

        def tw(t):
            return min(tile_w, m - t * tile_w)

        # dve_sem total: n_stream maxes + tail ops (s_all reduce + branch ops)
        n_dve = n_stream + 1 + (1 if k <= 1 else 7)

        @block.gpsimd
        def _(gpsimd):
            # s_y = x[p, y[p]] via one indirect gather
            gpsimd.dma_start(out=idx[:, :], in_=yoff[:, :]).then_inc(sw_sem, 16)
            gpsimd.wait_ge(sw_sem, 16)
            gpsimd.indirect_dma_start(
                out=sy[:, :],
                out_offset=None,
                in_=x[:, :],
                in_offset=bass.IndirectOffsetOnAxis(ap=idx[:, :1], axis=0),
            ).then_inc(sw_sem, 16)

        @block.sync
        def _(sync):
            for i in range(n_stream):
                t = i % n_tiles
                if i >= nbuf:
                    # slot reuse: exp(i-nbuf) must be done reading the slot;
                    # transitively this also covers the slot's previous DMA
                    sync.wait_ge(act_sem, i - nbuf + 2)
                s = (i % nbuf) * tile_w
                sync.dma_start(
                    out=xt[:, s : s + tw(t)],
                    in_=x2d[:, t * tile_w : t * tile_w + tw(t)],
                ).then_inc(dma_sems[i % nbuf], 16)
            # final store after Ln
            sync.wait_ge(act_sem, n_stream + 2)
            sync.dma_start(out=out[:, :], in_=loss[:, :]).then_inc(out_sem, 16)
            sync.wait_ge(out_sem, 16)

        @block.scalar
        def _(scalar):
            scalar.wait_ge(sw_sem, 32)
            nc.scalar.activation(
                out=neg_sy[:, :], in_=sy[:, :], func=copy_f, bias=0.0, scale=-1.0
            ).then_inc(act_sem, 1)
            # ACT is deep-pipelined: drain so the exps' bias read sees neg_sy
            scalar.wait_ge(act_sem, 1)
            for i in range(n_stream):
                t = i % n_tiles
                scalar.wait_ge(dma_sems[i % nbuf], 16 * (i // nbuf + 1))
                if i >= 2:
                    # escr ping-pong: max(i-2) must be done with this half
                    scalar.wait_ge(dve_sem, i - 1)
                e = (i % 2) * tile_w
                nc.scalar.activation(
                    out=escr[:, e : e + tw(t)],
                    in_=xt[:, (i % nbuf) * tile_w : (i % nbuf) * tile_w + tw(t)],
                    func=exp_f,
                    bias=neg_sy[:, :1],
                    scale=1.0,
                    accum_out=sums[:, t : t + 1],
                ).then_inc(act_sem, 1)
            scalar.wait_ge(dve_sem, n_dve)
            nc.scalar.activation(out=loss[:, :], in_=diff[:, :], func=ln_f).then_inc(
                act_sem, 1
            )

        @block.vector
        def _(vector):
            for i in range(n_stream):
                t = i % n_tiles
                vector.wait_ge(act_sem, i + 2)
                e = (i % 2) * tile_w
                nc.vector.max(
                    out=cand[:, 8 * t : 8 * t + 8], in_=escr[:, e : e + tw(t)]
                ).then_inc(dve_sem, 1)

            # Tail: DVE is pipelined too, so serialize each dependent step with
            # a retire-wait (tiny ops; sems are the only ordering primitive).
            cnt = n_stream

            def dve_op(emit):
                nonlocal cnt
                vector.wait_ge(dve_sem, cnt)
                cnt += 1
                emit().then_inc(dve_sem, 1)

            # all exps retired (max(n-1) waited act_sem >= n_tiles+1): sums ready
            dve_op(
                lambda: nc.vector.reduce_sum(
                    out=s_all[:, :], in_=sums[:, :], axis=mybir.AxisListType.X
                )
            )
            if k <= 1:
                # k=0: nothing excluded -> loss_row = log(S + 1)
                # k=1: only the label excluded -> log(S - 1 + 1) = log(S)
                if k == 0:
                    dve_op(
                        lambda: nc.vector.tensor_scalar_add(
                            diff[:, :], s_all[:, :], 1.0
                        )
                    )
                else:
                    dve_op(lambda: nc.vector.tensor_copy(diff[:, :], s_all[:, :]))
            else:
                dve_op(lambda: nc.vector.max(out=top8[:, :], in_=cand[:, :]))
                dve_op(lambda: nc.vector.memset(mlist[:, 0:1], 1.0))
                dve_op(lambda: nc.vector.memset(mlist[:, 1:8], SENTINEL))
                dve_op(
                    lambda: nc.vector.match_replace(
                        out=top8r[:, :],
                        in_to_replace=mlist[:, :],
                        in_values=top8[:, :],
                        imm_value=NEG_FILL,
                    )
                )
                dve_op(lambda: nc.vector.max(out=top8s[:, :], in_=top8r[:, :]))
                dve_op(
                    lambda: nc.vector.reduce_sum(
                        out=tsum[:, :],
                        in_=top8s[:, : k - 1],
                        axis=mybir.AxisListType.X,
                    )
                )
                dve_op(
                    lambda: nc.vector.tensor_sub(
                        out=diff[:, :], in0=s_all[:, :], in1=tsum[:, :]
                    )
                )
            assert cnt == n_dve, (cnt, n_dve)

    return nc


_program_cache: dict = {}


def _get_program(k: int) -> bass.Bass:
    if k not in _program_cache:
        _program_cache[k] = build_program(BP, M, k, TILE_W)
    return _program_cache[k]


def _run(x, y, k, **spmd_kwargs):
    x = np.asarray(x, dtype=np.float32)
    y = np.asarray(y)
    k = int(k)
    assert x.shape == (B, M), x.shape
    assert y.shape == (B,), y.shape

    nc = _get_program(k)
    in_maps = []
    for i in range(N_CORES):
        xs = np.ascontiguousarray(x[i * BP : (i + 1) * BP]).reshape(-1, 1)
        ys = y[i * BP : (i + 1) * BP].astype(np.int64)
        yo = (np.arange(BP, dtype=np.int64) * M + ys).astype(np.int32).reshape(BP, 1)
        in_maps.append({"x": xs, "yoff": yo})

    res = run_bass_kernel_spmd(nc, in_maps, list(range(N_CORES)), **spmd_kwargs)
    losses = np.concatenate(
        [np.asarray(r["out"], dtype=np.float32).reshape(BP) for r in res.results]
    )
    return np.asarray(losses.mean(dtype=np.float64), dtype=np.float32), res


def kernel(x, y, k) -> np.ndarray:
    out, _ = _run(x, y, k)
    return out
